# revision 14
# baseline (speedup 1.0000x reference)
"""Trainium2 Bass kernel for nn_Channel (complex FIR channel + FFT of taps).

Computation (per (n,p) row, N=4096, P=4):
  out  = complex_conv_full(x_row[567], cof_row[8])        -> 574 complex
  H    = FFT_64(zero-padded cof_row)                      -> 64 complex

Strategy:
  * Pure data parallel over 8 NeuronCores (512 n's -> 2048 rows each).
  * Rows on partitions (16 tiles of 128 rows per core). Complex data kept
    interleaved (re,im) along the free dim; a shift by 2l on the interleaved
    vector shifts both components by l.
  * Per-tap MAC passes: out += h_j * shift(src). Split between
      - TensorE: diagonal-weight matmuls accumulating in PSUM. All 16
        128x128 diagonal weights for a tile are built by ONE GPSIMD
        affine_select into a (128, 16*128) buffer (keep where col==row,
        fill 0). f32r (1 cyc/row) matmul inputs.
      - VectorE: fused scalar_tensor_tensor (per-partition scalar MAC)
        accumulating into the same PSUM region afterwards.
  * hi-taps need src = interleaved(-xi, xr); built once per tile on ScalarE
    with two strided copies.
  * H: one small matmul per tile: lhsT = cof^T chunk (16x128, host-prepped),
    rhs = 16x128 DFT re/im table -> PSUM (128 rows x 128) -> copy -> DMA.
"""

import os
import sys

import numpy as np

if "/opt/trn_rl_repo" not in sys.path:
    sys.path.insert(0, "/opt/trn_rl_repo")

# ---------------------------------------------------------------- constants
_N, _P, _SMK, _L, _M = 4096, 4, 567, 8, 64
_NCORES = 8
_ROWS = _N * _P // _NCORES          # 2048 rows per core
_NT = _ROWS // 128                  # 16 row-tiles per core
_WIN = _SMK * 2                     # 1134 interleaved input floats per row
_OUTW = (_SMK + _L - 1) * 2         # 1148 interleaved output floats per row
_PAD = 2 * (_L - 1)                 # 14 zero pad on each side
_PW = _PAD + _WIN + _PAD            # 1162 padded x width
_SKW = 144                          # skew buffer width (>= 127 + 16)
_NJ = 2 * _L                        # 16 taps (l, re/im)

# tap j = 2l + c  (c=0 -> hr applied to x; c=1 -> hi applied to swapneg(x))
_DVE_TAPS = (11, 12, 13, 14, 15)    # taps MAC'd on VectorE (fp16 SBUF acc)
_PE_TAPS = tuple(j for j in range(_NJ) if j not in _DVE_TAPS)
_CHUNKS = ((0, 512), (512, 1024), (1024, _OUTW))   # PSUM-bank aligned

_prog_cache = {}


def _build_program(repeat=1):
    import concourse.bacc as bacc
    import concourse.tile as tile
    from concourse import mybir

    f32 = mybir.dt.float32
    f16 = mybir.dt.float16

    nc = bacc.Bacc(None)
    xin = nc.declare_dram_parameter("xin", [_ROWS, _PW], f16, isOutput=False)
    cplh = nc.declare_dram_parameter("cplh", [_NT, 128, _NJ], f16, isOutput=False)
    cpl = nc.declare_dram_parameter("cpl", [_NT, 128, _NJ], f32, isOutput=False)
    coft = nc.declare_dram_parameter("coft", [_NJ, _ROWS], f32, isOutput=False)
    htab = nc.declare_dram_parameter("htab", [_NJ, 2 * _M], f32, isOutput=False)
    out = nc.declare_dram_parameter("out", [_ROWS, _OUTW], f32, isOutput=True)
    hout = nc.declare_dram_parameter("hout", [_ROWS, 2 * _M], f32, isOutput=True)

    from contextlib import ExitStack

    with tile.TileContext(nc) as tc, ExitStack() as ctx:
        singles = ctx.enter_context(tc.tile_pool(name="singles", bufs=1))
        xpool = ctx.enter_context(tc.tile_pool(name="xpool", bufs=3))
        xspool = ctx.enter_context(tc.tile_pool(name="xspool", bufs=3))
        wpool = ctx.enter_context(tc.tile_pool(name="wpool", bufs=3))
        cpool = ctx.enter_context(tc.tile_pool(name="cpool", bufs=3))
        opool = ctx.enter_context(tc.tile_pool(name="opool", bufs=3))
        hopool = ctx.enter_context(tc.tile_pool(name="hopool", bufs=3))
        pspool = ctx.enter_context(tc.tile_pool(name="psum", bufs=2, space="PSUM"))
        hpspool = ctx.enter_context(tc.tile_pool(name="hpsum", bufs=2, space="PSUM"))

        coft_t = singles.tile([_NJ, _ROWS], f32)
        nc.sync.dma_start(out=coft_t[:, :], in_=coft[:, :])
        htab_t = singles.tile([_NJ, 2 * _M], f32)
        nc.sync.dma_start(out=htab_t[:, :], in_=htab[:, :])

        rep_cm = tc.For_i(0, repeat, 1) if repeat > 1 else None
        if rep_cm is not None:
            rep_cm.__enter__()
        for t in range(_NT):
            r0 = t * 128
            # ---- loads
            xt = xpool.tile([128, _PW], f16)
            nc.sync.dma_start(out=xt[:, :], in_=xin[r0:r0 + 128, :])
            ct = cpool.tile([128, _NJ], f32)
            nc.sync.dma_start(out=ct[:, :], in_=cpl[t])
            cth = cpool.tile([128, _NJ], f16, tag="cth")
            nc.sync.dma_start(out=cth[:, :], in_=cplh[t])

            # all 16 diagonal weights in one GPSIMD op:
            # wt[p, j, m] = (p == m) ? h[p, j] : 0
            wt = wpool.tile([128, _NJ * 128], f16)
            nc.gpsimd.affine_select(
                out=wt[:, :].rearrange("p (j m) -> p j m", j=_NJ),
                in_=cth[:, :].unsqueeze(2).broadcast_to([128, _NJ, 128]),
                pattern=[[0, _NJ], [-1, 128]],
                compare_op=mybir.AluOpType.is_equal,
                fill=0.0,
                base=0,
                channel_multiplier=1,
            )

            # ---- xs = interleaved(-xi, xr), zero pads preserved
            xs = xspool.tile([128, _PW], f16)
            xt2 = xt[:, :].rearrange("p (n two) -> p n two", two=2)
            xs2 = xs[:, :].rearrange("p (n two) -> p n two", two=2)
            nc.scalar.mul(out=xs2[:, :, 0], in_=xt2[:, :, 1], mul=-1.0)
            nc.scalar.copy(out=xs2[:, :, 1], in_=xt2[:, :, 0])

            # ---- conv MACs into PSUM
            ps = pspool.tile([128, _OUTW], f32)
            first = True
            for j in _PE_TAPS:
                l, c = j // 2, j % 2
                src = xt if c == 0 else xs
                soff = _PAD - 2 * l
                last = j == _PE_TAPS[-1]
                for (c0, c1) in _CHUNKS:
                    nc.tensor.matmul(
                        ps[:, c0:c1],
                        lhsT=wt[:, 128 * j:128 * j + 128],
                        rhs=src[:, soff + c0:soff + c1],
                        start=first,
                        stop=last,
                    )
                first = False
            # DVE taps accumulate in an fp16 SBUF tile (2x/4x DVE modes),
            # then one TT-add folds psum + acc -> output tile.
            acc = xspool.tile([128, _OUTW], f16, tag="acc")
            for i, j in enumerate(_DVE_TAPS):
                l, c = j // 2, j % 2
                src = xt if c == 0 else xs
                soff = _PAD - 2 * l
                if i == 0:
                    nc.vector.tensor_scalar(
                        acc[:, :],
                        src[:, soff:soff + _OUTW],
                        ct[:, j:j + 1],
                        None,
                        op0=mybir.AluOpType.mult,
                    )
                else:
                    nc.vector.scalar_tensor_tensor(
                        out=acc[:, :],
                        in0=src[:, soff:soff + _OUTW],
                        scalar=ct[:, j:j + 1],
                        in1=acc[:, :],
                        op0=mybir.AluOpType.mult,
                        op1=mybir.AluOpType.add,
                    )

            # ---- combine psum + acc -> out tile, store
            ot = opool.tile([128, _OUTW], f32)
            nc.vector.tensor_tensor(
                out=ot[:, :],
                in0=ps[:, 0:_OUTW],
                in1=acc[:, :],
                op=mybir.AluOpType.add,
            )
            nc.sync.dma_start(out=out[r0:r0 + 128, :], in_=ot[:, :])

            # ---- H = cof^T chunk @ DFT table
            hps = hpspool.tile([128, 2 * _M], f32)
            nc.tensor.matmul(
                hps[:, :],
                lhsT=coft_t[:, r0:r0 + 128],
                rhs=htab_t[:, :],
                start=True,
                stop=True,
            )
            hot = hopool.tile([128, 2 * _M], f32)
            nc.scalar.copy(out=hot[:, :], in_=hps[:, :])
            nc.sync.dma_start(out=hout[r0:r0 + 128, :], in_=hot[:, :])

        if rep_cm is not None:
            rep_cm.__exit__(None, None, None)

    nc.compile()
    return nc


def _get_program(repeat=1):
    key = ("nc", repeat)
    if key not in _prog_cache:
        _prog_cache[key] = _build_program(repeat)
    return _prog_cache[key]


def _host_prep(x, cof):
    """Build per-core input maps from full inputs."""
    x = np.ascontiguousarray(x, dtype=np.float32)
    cof = np.ascontiguousarray(cof, dtype=np.float32)
    B = _N * _P
    x_flat = x.reshape(B, _WIN)
    h = cof.reshape(B, _NJ)                       # row: hr0 hi0 hr1 hi1 ...
    hr4 = h.reshape(_NCORES, _NT, 128, _NJ)

    # DFT table: htab[2l+c, 2k+c']
    l = np.arange(_L)[:, None]
    k = np.arange(_M)[None, :]
    th = 2.0 * np.pi * l * k / _M
    cos, sin = np.cos(th), np.sin(th)
    htab = np.zeros((_NJ, 2 * _M), dtype=np.float32)
    htab[0::2, 0::2] = cos          # hr -> re
    htab[0::2, 1::2] = -sin         # hr -> im
    htab[1::2, 0::2] = sin          # hi -> re
    htab[1::2, 1::2] = cos          # hi -> im

    x16 = np.zeros((B, _PW), dtype=np.float16)
    x16[:, _PAD:_PAD + _WIN] = x_flat
    h16 = hr4.astype(np.float16)
    in_maps = []
    for c in range(_NCORES):
        rows = slice(c * _ROWS, (c + 1) * _ROWS)
        in_maps.append({
            "xin": x16[rows],
            "cplh": h16[c],
            "cpl": hr4[c],
            "coft": np.ascontiguousarray(h[rows].T),
            "htab": htab,
        })
    return in_maps


def _run(in_maps, trace=False, repeat=1, **kw):
    from concourse.bass_utils import run_bass_kernel_spmd

    nc = _get_program(repeat)
    return run_bass_kernel_spmd(nc, in_maps, list(range(_NCORES)), trace=trace, **kw)


def kernel(x, cof, M):
    assert int(M) == _M, f"kernel hardcodes M=64, got {M}"
    in_maps = _host_prep(x, cof)
    res = _run(in_maps)
    outs = res.results
    conv = np.concatenate([r["out"] for r in outs], axis=0)
    hh = np.concatenate([r["hout"] for r in outs], axis=0)
    out = conv.reshape(_N, _P, _SMK + _L - 1, 2).astype(np.float32)
    H = hh.reshape(_N, _P, _M, 2).astype(np.float32)
    return out, H


# revision 15
# speedup vs baseline: 1.6775x; 1.6775x over previous
"""Trainium2 Bass kernel for nn_Channel (complex FIR channel + FFT of taps).

Computation (per (n,p) row, N=4096, P=4):
  out  = complex_conv_full(x_row[567], cof_row[8])        -> 574 complex
  H    = FFT_64(zero-padded cof_row)                      -> 64 complex

Strategy:
  * Pure data parallel over 8 NeuronCores (512 n's -> 2048 rows each).
  * Rows on partitions (16 tiles of 128 rows per core). Complex data kept
    interleaved (re,im) along the free dim; a shift by 2l on the interleaved
    vector shifts both components by l.
  * Per-tap MAC passes: out += h_j * shift(src). Split between
      - TensorE: diagonal-weight matmuls accumulating in PSUM. All 16
        128x128 diagonal weights for a tile are built by ONE GPSIMD
        affine_select into a (128, 16*128) buffer (keep where col==row,
        fill 0). f32r (1 cyc/row) matmul inputs.
      - VectorE: fused scalar_tensor_tensor (per-partition scalar MAC)
        accumulating into the same PSUM region afterwards.
  * hi-taps need src = interleaved(-xi, xr); built once per tile on ScalarE
    with two strided copies.
  * H: one small matmul per tile: lhsT = cof^T chunk (16x128, host-prepped),
    rhs = 16x128 DFT re/im table -> PSUM (128 rows x 128) -> copy -> DMA.
"""

import os
import sys

import numpy as np

if "/opt/trn_rl_repo" not in sys.path:
    sys.path.insert(0, "/opt/trn_rl_repo")

# ---------------------------------------------------------------- constants
_N, _P, _SMK, _L, _M = 4096, 4, 567, 8, 64
_NCORES = 8
_ROWS = _N * _P // _NCORES          # 2048 rows per core
_NT = _ROWS // 128                  # 16 row-tiles per core
_WIN = _SMK * 2                     # 1134 interleaved input floats per row
_OUTW = (_SMK + _L - 1) * 2         # 1148 interleaved output floats per row
_PAD = 2 * (_L - 1)                 # 14 zero pad on each side
_PW = _PAD + _WIN + _PAD            # 1162 padded x width
_SKW = 144                          # skew buffer width (>= 127 + 16)
_NJ = 2 * _L                        # 16 taps (l, re/im)

# tap j = 2l + c  (c=0 -> hr applied to x; c=1 -> hi applied to swapneg(x))
_DVE_TAPS = (12, 13, 14, 15)        # taps MAC'd on VectorE (into PSUM)
_PE_TAPS = tuple(j for j in range(_NJ) if j not in _DVE_TAPS)
_CHUNKS = ((0, 512), (512, 1024), (1024, _OUTW))   # PSUM-bank aligned

_prog_cache = {}


def _build_program(repeat=1):
    import concourse.bacc as bacc
    import concourse.tile as tile
    from concourse import mybir

    f32 = mybir.dt.float32
    f16 = mybir.dt.float16

    nc = bacc.Bacc(None)
    xin = nc.declare_dram_parameter("xin", [_ROWS, _PW], f16, isOutput=False)
    cplh = nc.declare_dram_parameter("cplh", [_NT, 128, _NJ], f16, isOutput=False)
    cpl = nc.declare_dram_parameter("cpl", [_NT, 128, _NJ], f32, isOutput=False)
    coft = nc.declare_dram_parameter("coft", [_NJ, _ROWS], f32, isOutput=False)
    htab = nc.declare_dram_parameter("htab", [_NJ, 2 * _M], f32, isOutput=False)
    out = nc.declare_dram_parameter("out", [_ROWS, _OUTW], f32, isOutput=True)
    hout = nc.declare_dram_parameter("hout", [_ROWS, 2 * _M], f32, isOutput=True)

    from contextlib import ExitStack

    with tile.TileContext(nc) as tc, ExitStack() as ctx:
        singles = ctx.enter_context(tc.tile_pool(name="singles", bufs=1))
        xpool = ctx.enter_context(tc.tile_pool(name="xpool", bufs=3))
        xspool = ctx.enter_context(tc.tile_pool(name="xspool", bufs=3))
        wpool = ctx.enter_context(tc.tile_pool(name="wpool", bufs=3))
        cpool = ctx.enter_context(tc.tile_pool(name="cpool", bufs=3))
        opool = ctx.enter_context(tc.tile_pool(name="opool", bufs=3))
        hopool = ctx.enter_context(tc.tile_pool(name="hopool", bufs=3))
        pspool = ctx.enter_context(tc.tile_pool(name="psum", bufs=2, space="PSUM"))
        hpspool = ctx.enter_context(tc.tile_pool(name="hpsum", bufs=2, space="PSUM"))

        coft_t = singles.tile([_NJ, _ROWS], f32)
        nc.sync.dma_start(out=coft_t[:, :], in_=coft[:, :])
        htab_t = singles.tile([_NJ, 2 * _M], f32)
        nc.sync.dma_start(out=htab_t[:, :], in_=htab[:, :])

        rep_cm = (tc.For_i(0, repeat, 1, hint_engines=(
            mybir.EngineType.PE, mybir.EngineType.SP, mybir.EngineType.DVE,
            mybir.EngineType.Activation, mybir.EngineType.Pool))
            if repeat > 1 else None)
        if rep_cm is not None:
            rep_cm.__enter__()
        for t in range(_NT):
            r0 = t * 128
            # ---- loads
            xt = xpool.tile([128, _PW], f16)
            nc.sync.dma_start(out=xt[:, :], in_=xin[r0:r0 + 128, :])
            ct = cpool.tile([128, _NJ], f32)
            nc.sync.dma_start(out=ct[:, :], in_=cpl[t])
            cth = cpool.tile([128, _NJ], f16, tag="cth")
            nc.sync.dma_start(out=cth[:, :], in_=cplh[t])

            # all 16 diagonal weights in one GPSIMD op:
            # wt[p, j, m] = (p == m) ? h[p, j] : 0
            wt = wpool.tile([128, _NJ * 128], f16)
            nc.gpsimd.affine_select(
                out=wt[:, :].rearrange("p (j m) -> p j m", j=_NJ),
                in_=cth[:, :].unsqueeze(2).broadcast_to([128, _NJ, 128]),
                pattern=[[0, _NJ], [-1, 128]],
                compare_op=mybir.AluOpType.is_equal,
                fill=0.0,
                base=0,
                channel_multiplier=1,
            )

            # ---- xs = interleaved(-xi, xr), zero pads preserved
            xs = xspool.tile([128, _PW], f16)
            xt2 = xt[:, :].rearrange("p (n two) -> p n two", two=2)
            xs2 = xs[:, :].rearrange("p (n two) -> p n two", two=2)
            nc.scalar.mul(out=xs2[:, :, 0], in_=xt2[:, :, 1], mul=-1.0)
            nc.scalar.copy(out=xs2[:, :, 1], in_=xt2[:, :, 0])

            # ---- conv MACs into PSUM
            ps = pspool.tile([128, _OUTW], f32)
            first = True
            for j in _PE_TAPS:
                l, c = j // 2, j % 2
                src = xt if c == 0 else xs
                soff = _PAD - 2 * l
                last = j == _PE_TAPS[-1]
                for (c0, c1) in _CHUNKS:
                    nc.tensor.matmul(
                        ps[:, c0:c1],
                        lhsT=wt[:, 128 * j:128 * j + 128],
                        rhs=src[:, soff + c0:soff + c1],
                        start=first,
                        stop=last,
                    )
                first = False
            for j in _DVE_TAPS:
                l, c = j // 2, j % 2
                src = xt if c == 0 else xs
                soff = _PAD - 2 * l
                nc.vector.scalar_tensor_tensor(
                    out=ps[:, 0:_OUTW],
                    in0=src[:, soff:soff + _OUTW],
                    scalar=ct[:, j:j + 1],
                    in1=ps[:, 0:_OUTW],
                    op0=mybir.AluOpType.mult,
                    op1=mybir.AluOpType.add,
                )

            # ---- copy out + store
            ot = opool.tile([128, _OUTW], f32)
            nc.scalar.copy(out=ot[:, :], in_=ps[:, 0:_OUTW])
            nc.sync.dma_start(out=out[r0:r0 + 128, :], in_=ot[:, :])

            # ---- H = cof^T chunk @ DFT table
            hps = hpspool.tile([128, 2 * _M], f32)
            nc.tensor.matmul(
                hps[:, :],
                lhsT=coft_t[:, r0:r0 + 128],
                rhs=htab_t[:, :],
                start=True,
                stop=True,
            )
            hot = hopool.tile([128, 2 * _M], f32)
            nc.scalar.copy(out=hot[:, :], in_=hps[:, :])
            nc.sync.dma_start(out=hout[r0:r0 + 128, :], in_=hot[:, :])

        if rep_cm is not None:
            rep_cm.__exit__(None, None, None)

    nc.compile()
    return nc


def _get_program(repeat=1):
    key = ("nc", repeat)
    if key not in _prog_cache:
        _prog_cache[key] = _build_program(repeat)
    return _prog_cache[key]


def _host_prep(x, cof):
    """Build per-core input maps from full inputs."""
    x = np.ascontiguousarray(x, dtype=np.float32)
    cof = np.ascontiguousarray(cof, dtype=np.float32)
    B = _N * _P
    x_flat = x.reshape(B, _WIN)
    h = cof.reshape(B, _NJ)                       # row: hr0 hi0 hr1 hi1 ...
    hr4 = h.reshape(_NCORES, _NT, 128, _NJ)

    # DFT table: htab[2l+c, 2k+c']
    l = np.arange(_L)[:, None]
    k = np.arange(_M)[None, :]
    th = 2.0 * np.pi * l * k / _M
    cos, sin = np.cos(th), np.sin(th)
    htab = np.zeros((_NJ, 2 * _M), dtype=np.float32)
    htab[0::2, 0::2] = cos          # hr -> re
    htab[0::2, 1::2] = -sin         # hr -> im
    htab[1::2, 0::2] = sin          # hi -> re
    htab[1::2, 1::2] = cos          # hi -> im

    x16 = np.zeros((B, _PW), dtype=np.float16)
    x16[:, _PAD:_PAD + _WIN] = x_flat
    h16 = hr4.astype(np.float16)
    in_maps = []
    for c in range(_NCORES):
        rows = slice(c * _ROWS, (c + 1) * _ROWS)
        in_maps.append({
            "xin": x16[rows],
            "cplh": h16[c],
            "cpl": hr4[c],
            "coft": np.ascontiguousarray(h[rows].T),
            "htab": htab,
        })
    return in_maps


def _run(in_maps, trace=False, repeat=1, **kw):
    from concourse.bass_utils import run_bass_kernel_spmd

    nc = _get_program(repeat)
    return run_bass_kernel_spmd(nc, in_maps, list(range(_NCORES)), trace=trace, **kw)


def kernel(x, cof, M):
    assert int(M) == _M, f"kernel hardcodes M=64, got {M}"
    in_maps = _host_prep(x, cof)
    res = _run(in_maps)
    outs = res.results
    conv = np.concatenate([r["out"] for r in outs], axis=0)
    hh = np.concatenate([r["hout"] for r in outs], axis=0)
    out = conv.reshape(_N, _P, _SMK + _L - 1, 2).astype(np.float32)
    H = hh.reshape(_N, _P, _M, 2).astype(np.float32)
    return out, H
